# revision 28
# baseline (speedup 1.0000x reference)
"""Trainium2 Bass kernel for the ContinuousRNN problem.

Reference computation (per batch row b):
    h_0 = 0                         # [N], N=100
    z_t = W_rec @ h_t + W_in @ u_t  # u_t = inputs[b, t] (3-dim)
    h_{t+1} = (1-DT)*h_t + DT*tanh(z_t) + NOISE_STD*noise[b, t]
    out_t = W_out @ h_{t+1}         # 3-dim

Strategy: data-parallel over batch across 8 NeuronCores (64 rows/core).
On-core state is h^T [100 partitions, 64 batch cols] living in an fp16
SBUF ring.  One fp16 matmul per step with stationary
    S.T = [[W_rec, W_in], [W_out, 0]]   (padded to 128x128 fp16)
computes z_t, the input projection (u rides in rhs rows 100:103 of the
h-ring, DMA'd in bulk as fp16), and the output projection W_out@h_t
(rides in psum rows 100:103) in a single single-pass PE instruction
(fp16 -> no fp32 LOW/HIGH double pass, FWL-eligible 128-col stationary).
tanh on the scalar engine (PSUM -> PSUM by default, the cheapest
ACTIVATE form; th_psum=False switches to PSUM -> SBUF fp16), the
two-term affine update on DVE: a = 0.85*h + scaled-noise (all-SBUF,
off the critical path) and h' = 0.15*th + a -> next fp16 ring slot.
Outputs are block-copied from psum rows 100:103 to SBUF (alternating
ACT/DVE) and DMA'd to DRAM.

The steady state is bound by the serial per-step chain
MATMUL -> ACTIVATE -> STT -> MATMUL (~800ns on HW) with two batch
groups hiding engine idle time.  Measured dead ends: 3 unequal groups
(ACT does not pipeline below its instruction duration), and a z-space
2-hop form where PE start=False accumulates over DVE-written PSUM --
CoreSim models that as accumulation but real HW overwrites.

Host side does layout only: batch shard, (N,T,B)/(3,T,B) transposes,
NOISE_STD pre-scale, fp16 casts, and packing of the padded stationary.
"""

import sys

for _p in ("/opt/trn_rl_repo",):
    if _p not in sys.path:
        sys.path.insert(0, _p)

import numpy as np

import concourse.bass as bass
import concourse.bacc as bacc
import concourse.mybir as mybir
from concourse import tile
from concourse.bass_utils import run_bass_kernel_spmd

F32 = mybir.dt.float32
F16 = mybir.dt.float16

N = 100          # hidden size
NB = 3           # n_bits
K = N + NB       # live matmul rows (103)
KP = 128         # padded contraction/output size
B = 512          # full batch
T = 2048         # time steps
NCORES = 8
BL = B // NCORES  # batch per core (64)
DT = np.float32(0.15)
NOISE_STD = np.float32(0.015)
DECAY = np.float32(1.0) - DT  # 0.85


def emit_rnn(tc, nc, aps, *, t_steps=T, bl=BL, groups=2,
             tc_chunk=128, ring_slots=128, z_slots=16, th_slots=8, a_slots=4,
             th_psum=False):
    """Emit the unrolled RNN scan.

    aps: dict with DRAM APs: s_mat [KP,KP] fp16, noise_t [N, t_steps*bl],
         u_t [NB, t_steps*bl] fp16, out_t [NB, (t_steps+1)*bl].
    """
    assert t_steps % tc_chunk == 0
    ub = ring_slots // 2          # u-block size in steps (half the ring)
    assert t_steps % ub == 0
    # group widths in batch cols (unequal when groups doesn't divide bl)
    gws = [bl // groups + (1 if g < bl % groups else 0)
           for g in range(groups)]
    gos = [sum(gws[:g]) for g in range(groups)]  # group col offsets
    mult = mybir.AluOpType.mult
    add = mybir.AluOpType.add
    tanh = mybir.ActivationFunctionType.Tanh

    cpool = tc.alloc_tile_pool(name="const", bufs=1)
    rpool = tc.alloc_tile_pool(name="ring", bufs=1)
    npool = tc.alloc_tile_pool(name="noise", bufs=2)
    spool = tc.alloc_tile_pool(name="stage", bufs=2)
    ppool = tc.alloc_tile_pool(name="psum", bufs=1, space="PSUM")

    # Stationary weights [KP, KP] fp16 (padded; full 128 cols -> FWL)
    s_sb = cpool.tile([KP, KP], F16, name="s_sb")
    nc.sync.dma_start(s_sb[:, :], aps["s_mat"][:, :])
    # Second stationary DT*[[W],[Wo]] for the th-term matmul
    s2_sb = cpool.tile([KP, KP], F16, name="s2_sb")
    nc.sync.dma_start(s2_sb[:, :], aps["s2_mat"][:, :])

    # h/u ring (fp16): rows 0:N = h state, rows N:K = u inputs
    # (DMA-filled), rows K:KP = zero padding for the 128-row contraction
    ring = rpool.tile([KP, ring_slots * bl], F16, name="ring")
    # tanh-output ring per group, SBUF fp16 (PE rhs for the th matmul);
    # rows N:KP are zero padding for the 128-row contraction
    th_sb = [rpool.tile([KP, th_slots * gws[g]], F16, name=f"th_sb{g}")
             for g in range(groups)]
    # fp32 intermediate q = 0.85*a_{t-1} + noise', shared across groups
    # (one full-width DVE op per step: the ring cols are adjacent)
    a_ring = rpool.tile([N, a_slots * bl], F32, name="a_ring")

    # PSUM: two z regions per group (double-buffered for output drain)
    zps = [[ppool.tile([KP, z_slots * gws[g]], F32, name=f"zps{g}_{r}")
            for r in range(2)] for g in range(groups)]

    # Zero rows 96:KP of the whole ring once: covers the K:KP contraction
    # padding for every slot (u DMA later refills rows N:K per block).
    nc.vector.memset(ring[96:KP, :], 0.0)
    # a_{-1} = 0 (ring slot s holds [a_{s-1}; u_s])
    nc.vector.memset(ring[0:N, 0:bl], 0.0)
    # th padding rows (rows N:KP of th rings never written by ACT)
    for g in range(groups):
        nc.vector.memset(th_sb[g][96:KP, :], 0.0)

    n_ublocks = t_steps // ub

    def u_dma(k):
        if k >= n_ublocks:
            return
        half = (k % 2) * ub
        nc.sync.dma_start(
            ring[N:K, half * bl:(half + ub) * bl],
            aps["u_t"][:, k * ub * bl:(k + 1) * ub * bl])

    u_dma(0)

    nz_tiles = {}

    def noise_tile(c):
        if c * tc_chunk >= t_steps:
            return None
        if c not in nz_tiles:
            tl = npool.tile([N, tc_chunk * bl], F32, tag="nz", name=f"nz{c}")
            nc.sync.dma_start(
                tl[:, :],
                aps["noise_t"][:, c * tc_chunk * bl:(c + 1) * tc_chunk * bl])
            nz_tiles[c] = tl
        return nz_tiles[c]

    noise_tile(0)

    n_steps = t_steps + 1  # one extra matmul for the final output row
    for t in range(n_steps):
        slot = t % ring_slots
        if t < t_steps and t % tc_chunk == 0:
            c = t // tc_chunk
            noise_tile(c + 1)
            nzt = nz_tiles[c]
        if t < t_steps and t % ub == 0:
            # program order matters: block k+1 overwrites the half that
            # block k-1 reads, so it must be emitted after those reads
            u_dma(t // ub + 1)

        zslot = t % z_slots
        reg = (t // z_slots) % 2
        tt = t % tc_chunk

        if t < t_steps:
            asl = t % a_slots
            nslot = (t + 1) % ring_slots
            if t == 0:
                # a_0 = q_0 = 0.85*a_{-1} + n_0 (th_{-1} = 0), both groups
                nc.vector.scalar_tensor_tensor(
                    ring[0:N, nslot * bl:nslot * bl + bl],
                    ring[0:N, slot * bl:slot * bl + bl],
                    float(DECAY),
                    nzt[:, tt * bl:(tt + 1) * bl],
                    mult, add)
            else:
                # q_t = 0.85*a_{t-1} + n_t (joint 64-col DVE op, off-chain)
                nc.vector.scalar_tensor_tensor(
                    a_ring[:, asl * bl:(asl + 1) * bl],
                    ring[0:N, slot * bl:slot * bl + bl],
                    float(DECAY),
                    nzt[:, tt * bl:(tt + 1) * bl],
                    mult, add)

        for g in range(groups):
            gw = gws[g]
            c0, c1 = gos[g], gos[g] + gw
            zp = zps[g][reg]
            # z_t = W@a_{t-1} + U@u_t (+ out rows): a/u ride the ring slot
            nc.tensor.matmul(
                zp[:, zslot * gw:(zslot + 1) * gw],
                s_sb[:, :],
                ring[:, slot * bl + c0:slot * bl + c1],
                start=True, stop=(t == 0), skip_group_check=True)
            if t > 0:
                # z_t += DT*W@th_{t-1} (+ out rows); PE-over-PE accumulate
                pths = (t - 1) % th_slots
                nc.tensor.matmul(
                    zp[:, zslot * gw:(zslot + 1) * gw],
                    s2_sb[:, :],
                    th_sb[g][:, pths * gw:(pths + 1) * gw],
                    start=False, stop=True, skip_group_check=True)

            if t < t_steps:
                ths = t % th_slots
                # tanh (ACT, psum -> sbuf fp16): th_t
                nc.scalar.activation(
                    th_sb[g][0:N, ths * gw:(ths + 1) * gw],
                    zp[0:N, zslot * gw:(zslot + 1) * gw],
                    tanh)
                if t > 0:
                    # a_t = 0.85*DT*th_{t-1} + q_t -> next ring slot
                    pths = (t - 1) % th_slots
                    nc.vector.scalar_tensor_tensor(
                        ring[0:N, nslot * bl + c0:nslot * bl + c1],
                        th_sb[g][0:N, pths * gw:(pths + 1) * gw],
                        float(DECAY * DT),
                        a_ring[:, asl * bl + c0:asl * bl + c1],
                        mult, add)

        # Output drain: retire a z region once its last slot is written.
        if t % z_slots == z_slots - 1 or t == n_steps - 1:
            r_idx = t // z_slots
            nslots = (t % z_slots) + 1
            for g in range(groups):
                gw = gws[g]
                zp = zps[g][reg]
                # compute-engine APs must start on a 32-aligned
                # partition: copy rows 96:103, DMA out only 100:103
                stg = spool.tile([K - 96, z_slots * gw], F32, tag=f"st{g}",
                                 name=f"stg{g}_{r_idx}")
                src = zp[96:K, 0:nslots * gw]
                dst = stg[:, 0:nslots * gw]
                # always DVE: the ACT engine is on the 2-hop critical chain
                nc.vector.tensor_copy(dst, src)
                # DRAM layout: col block j (width bl) = W_out @ h_j;
                # group g owns cols j*bl + [gos[g], gos[g]+gw).
                base = (r_idx * z_slots)
                nc.sync.dma_start(
                    aps["out_t"].rearrange("p (t b) -> p t b", b=bl)[
                        :, base:base + nslots, gos[g]:gos[g] + gw],
                    stg.rearrange("p (t b) -> p t b", b=gw)[
                        N - 96:K - 96, 0:nslots, :])

    for p in (ppool, spool, npool, rpool, cpool):
        p.release()


def build_nc(*, t_steps=T, bl=BL, groups=2, tc_chunk=128, ring_slots=128,
             z_slots=16, th_slots=8, a_slots=4, num_devices=NCORES,
             th_psum=True):
    nc = bacc.Bacc("TRN2", target_bir_lowering=False, debug=False,
                   num_devices=num_devices)
    aps = {
        "s_mat": nc.dram_tensor("s_mat", [KP, KP], F16,
                                kind="ExternalInput").ap(),
        "s2_mat": nc.dram_tensor("s2_mat", [KP, KP], F16,
                                 kind="ExternalInput").ap(),
        "noise_t": nc.dram_tensor("noise_t", [N, t_steps * bl], F32,
                                  kind="ExternalInput").ap(),
        "u_t": nc.dram_tensor("u_t", [NB, t_steps * bl], F16,
                              kind="ExternalInput").ap(),
        "out_t": nc.dram_tensor("out_t", [NB, (t_steps + 1) * bl], F32,
                                kind="ExternalOutput").ap(),
    }
    with tile.TileContext(nc) as tcx:
        emit_rnn(tcx, nc, aps, t_steps=t_steps, bl=bl, groups=groups,
                 tc_chunk=tc_chunk, ring_slots=ring_slots, z_slots=z_slots,
                 th_slots=th_slots, a_slots=a_slots, th_psum=th_psum)
    nc.compile()
    return nc


def make_s_mat(recurrent_weights, input_weights, output_weights):
    st = np.zeros((K, K), np.float32)
    st[:N, :N] = recurrent_weights
    st[:N, N:] = input_weights
    st[N:, :N] = output_weights
    sp = np.zeros((KP, KP), np.float16)
    sp[:K, :K] = st.T.astype(np.float16)
    return np.ascontiguousarray(sp)


def make_s2_mat(recurrent_weights, output_weights):
    st = np.zeros((K, K), np.float32)
    st[:N, :N] = recurrent_weights
    st[N:, :N] = output_weights
    sp = np.zeros((KP, KP), np.float16)
    sp[:K, :K] = (DT * st).T.astype(np.float16)
    return np.ascontiguousarray(sp)


def make_in_maps(inputs, noise, recurrent_weights, input_weights,
                 output_weights, *, t_steps=T, bl=BL, ncores=NCORES):
    s = make_s_mat(recurrent_weights, input_weights, output_weights)
    s2 = make_s2_mat(recurrent_weights, output_weights)
    in_maps = []
    for c in range(ncores):
        bs = slice(c * bl, (c + 1) * bl)
        nt = (noise[bs].astype(np.float32).transpose(2, 1, 0)
              * NOISE_STD).reshape(N, t_steps * bl)
        ut = np.ascontiguousarray(
            inputs[bs].astype(np.float16).transpose(2, 1, 0)
        ).reshape(NB, t_steps * bl)
        in_maps.append({"s_mat": s, "s2_mat": s2,
                        "noise_t": np.ascontiguousarray(nt),
                        "u_t": ut})
    return in_maps


def gather_out(results, *, t_steps=T, bl=BL, ncores=NCORES):
    out = np.empty((ncores * bl, t_steps, NB), np.float32)
    for c in range(ncores):
        ot = results[c]["out_t"].reshape(NB, t_steps + 1, bl)
        out[c * bl:(c + 1) * bl] = ot[:, 1:, :].transpose(2, 1, 0)
    return out


_NC_CACHE = {}


def kernel(inputs, noise, recurrent_weights, input_weights, output_weights,
           **run_kwargs):
    cfg = run_kwargs.pop("cfg", {})
    key = tuple(sorted(cfg.items()))
    if key not in _NC_CACHE:
        _NC_CACHE[key] = build_nc(**cfg)
    nc = _NC_CACHE[key]
    in_maps = make_in_maps(inputs, noise, recurrent_weights, input_weights,
                           output_weights)
    res = run_bass_kernel_spmd(nc, in_maps, core_ids=list(range(NCORES)),
                               **run_kwargs)
    out = gather_out(res.results)
    if run_kwargs.get("trace"):
        return out, res
    return out


# revision 29
# speedup vs baseline: 1.0843x; 1.0843x over previous
"""Trainium2 Bass kernel for the ContinuousRNN problem.

Reference computation (per batch row b):
    h_0 = 0                         # [N], N=100
    z_t = W_rec @ h_t + W_in @ u_t  # u_t = inputs[b, t] (3-dim)
    h_{t+1} = (1-DT)*h_t + DT*tanh(z_t) + NOISE_STD*noise[b, t]
    out_t = W_out @ h_{t+1}         # 3-dim

Strategy: data-parallel over batch across 8 NeuronCores (64 rows/core).
On-core state is h^T [100 partitions, 64 batch cols] living in an fp16
SBUF ring.  One fp16 matmul per step with stationary
    S.T = [[W_rec, W_in], [W_out, 0]]   (padded to 128x128 fp16)
computes z_t, the input projection (u rides in rhs rows 100:103 of the
h-ring, DMA'd in bulk as fp16), and the output projection W_out@h_t
(rides in psum rows 100:103) in a single single-pass PE instruction
(fp16 -> no fp32 LOW/HIGH double pass, FWL-eligible 128-col stationary).
tanh on the scalar engine (PSUM -> PSUM by default, the cheapest
ACTIVATE form; th_psum=False switches to PSUM -> SBUF fp16), the
two-term affine update on DVE: a = 0.85*h + scaled-noise (all-SBUF,
off the critical path) and h' = 0.15*th + a -> next fp16 ring slot.
Outputs are block-copied from psum rows 100:103 to SBUF (alternating
ACT/DVE) and DMA'd to DRAM.

The steady state is bound by the serial per-step chain
MATMUL -> ACTIVATE -> STT -> MATMUL (~800ns on HW) with two batch
groups hiding engine idle time.  Measured dead ends: 3 unequal groups
(ACT does not pipeline below its instruction duration), and a z-space
2-hop form where PE start=False accumulates over DVE-written PSUM --
CoreSim models that as accumulation but real HW overwrites.

Host side does layout only: batch shard, (N,T,B)/(3,T,B) transposes,
NOISE_STD pre-scale, fp16 casts, and packing of the padded stationary.
"""

import sys

for _p in ("/opt/trn_rl_repo",):
    if _p not in sys.path:
        sys.path.insert(0, _p)

import numpy as np

import concourse.bass as bass
import concourse.bacc as bacc
import concourse.mybir as mybir
from concourse import tile
from concourse.bass_utils import run_bass_kernel_spmd

F32 = mybir.dt.float32
F16 = mybir.dt.float16

N = 100          # hidden size
NB = 3           # n_bits
K = N + NB       # live matmul rows (103)
KP = 128         # padded contraction/output size
B = 512          # full batch
T = 2048         # time steps
NCORES = 8
BL = B // NCORES  # batch per core (64)
DT = np.float32(0.15)
NOISE_STD = np.float32(0.015)
DECAY = np.float32(1.0) - DT  # 0.85


def emit_rnn(tc, nc, aps, *, t_steps=T, bl=BL, groups=2,
             tc_chunk=128, ring_slots=128, z_slots=16, th_slots=8, a_slots=4,
             th_psum=False):
    """Emit the unrolled RNN scan.

    aps: dict with DRAM APs: s_mat [KP,KP] fp16, noise_t [N, t_steps*bl],
         u_t [NB, t_steps*bl] fp16, out_t [NB, (t_steps+1)*bl].
    """
    assert t_steps % tc_chunk == 0
    ub = ring_slots // 2          # u-block size in steps (half the ring)
    assert t_steps % ub == 0
    # group widths in batch cols (unequal when groups doesn't divide bl)
    gws = [bl // groups + (1 if g < bl % groups else 0)
           for g in range(groups)]
    gos = [sum(gws[:g]) for g in range(groups)]  # group col offsets
    mult = mybir.AluOpType.mult
    add = mybir.AluOpType.add
    tanh = mybir.ActivationFunctionType.Tanh

    cpool = tc.alloc_tile_pool(name="const", bufs=1)
    rpool = tc.alloc_tile_pool(name="ring", bufs=1)
    npool = tc.alloc_tile_pool(name="noise", bufs=2)
    spool = tc.alloc_tile_pool(name="stage", bufs=2)
    ppool = tc.alloc_tile_pool(name="psum", bufs=1, space="PSUM")

    # Stationary weights [KP, KP] fp16 (padded; full 128 cols -> FWL)
    s_sb = cpool.tile([KP, KP], F16, name="s_sb")
    nc.sync.dma_start(s_sb[:, :], aps["s_mat"][:, :])
    # Second stationary DT*[[W],[Wo]] for the th-term matmul
    s2_sb = cpool.tile([KP, KP], F16, name="s2_sb")
    nc.sync.dma_start(s2_sb[:, :], aps["s2_mat"][:, :])

    # h/u ring (fp16): rows 0:N = h state, rows N:K = u inputs
    # (DMA-filled), rows K:KP = zero padding for the 128-row contraction
    ring = rpool.tile([KP, ring_slots * bl], F16, name="ring")
    # tanh-output ring per group, SBUF fp16 (PE rhs for the th matmul);
    # rows N:KP are zero padding for the 128-row contraction
    th_sb = [rpool.tile([KP, th_slots * gws[g]], F16, name=f"th_sb{g}")
             for g in range(groups)]
    # fp32 intermediate q = 0.85*a_{t-1} + noise' per group
    a_ring = [rpool.tile([N, a_slots * gws[g]], F32, name=f"a_ring{g}")
              for g in range(groups)]

    # PSUM: two z regions per group (double-buffered for output drain)
    zps = [[ppool.tile([KP, z_slots * gws[g]], F32, name=f"zps{g}_{r}")
            for r in range(2)] for g in range(groups)]

    # Zero rows 96:KP of the whole ring once: covers the K:KP contraction
    # padding for every slot (u DMA later refills rows N:K per block).
    nc.vector.memset(ring[96:KP, :], 0.0)
    # a_{-1} = 0 (ring slot s holds [a_{s-1}; u_s])
    nc.vector.memset(ring[0:N, 0:bl], 0.0)
    # th padding rows (rows N:KP of th rings never written by ACT)
    for g in range(groups):
        nc.vector.memset(th_sb[g][96:KP, :], 0.0)

    n_ublocks = t_steps // ub

    def u_dma(k):
        if k >= n_ublocks:
            return
        half = (k % 2) * ub
        nc.sync.dma_start(
            ring[N:K, half * bl:(half + ub) * bl],
            aps["u_t"][:, k * ub * bl:(k + 1) * ub * bl])

    u_dma(0)

    nz_tiles = {}

    def noise_tile(c):
        if c * tc_chunk >= t_steps:
            return None
        if c not in nz_tiles:
            tl = npool.tile([N, tc_chunk * bl], F32, tag="nz", name=f"nz{c}")
            nc.sync.dma_start(
                tl[:, :],
                aps["noise_t"][:, c * tc_chunk * bl:(c + 1) * tc_chunk * bl])
            nz_tiles[c] = tl
        return nz_tiles[c]

    noise_tile(0)

    n_steps = t_steps + 1  # one extra matmul for the final output row
    for t in range(n_steps):
        slot = t % ring_slots
        if t < t_steps and t % tc_chunk == 0:
            c = t // tc_chunk
            noise_tile(c + 1)
            nzt = nz_tiles[c]
        if t < t_steps and t % ub == 0:
            # program order matters: block k+1 overwrites the half that
            # block k-1 reads, so it must be emitted after those reads
            u_dma(t // ub + 1)

        zslot = t % z_slots
        reg = (t // z_slots) % 2
        tt = t % tc_chunk

        for g in range(groups):
            gw = gws[g]
            c0, c1 = gos[g], gos[g] + gw
            zp = zps[g][reg]
            # z_t = W@a_{t-1} + U@u_t (+ out rows): a/u ride the ring slot
            nc.tensor.matmul(
                zp[:, zslot * gw:(zslot + 1) * gw],
                s_sb[:, :],
                ring[:, slot * bl + c0:slot * bl + c1],
                start=True, stop=(t == 0), skip_group_check=True)
            if t > 0:
                # z_t += DT*W@th_{t-1} (+ out rows); PE-over-PE accumulate
                pths = (t - 1) % th_slots
                nc.tensor.matmul(
                    zp[:, zslot * gw:(zslot + 1) * gw],
                    s2_sb[:, :],
                    th_sb[g][:, pths * gw:(pths + 1) * gw],
                    start=False, stop=True, skip_group_check=True)

            if t < t_steps:
                ths = t % th_slots
                # tanh (ACT, psum -> sbuf fp16): th_t
                nc.scalar.activation(
                    th_sb[g][0:N, ths * gw:(ths + 1) * gw],
                    zp[0:N, zslot * gw:(zslot + 1) * gw],
                    tanh)
                asl = t % a_slots
                nslot = (t + 1) % ring_slots
                if t == 0:
                    # a_0 = q_0 = 0.85*a_{-1} + n_0 (th_{-1} = 0)
                    nc.vector.scalar_tensor_tensor(
                        ring[0:N, nslot * bl + c0:nslot * bl + c1],
                        ring[0:N, slot * bl + c0:slot * bl + c1],
                        float(DECAY),
                        nzt[:, tt * bl + c0:tt * bl + c1],
                        mult, add)
                else:
                    # q_t = 0.85*a_{t-1} + n_t (DVE, off the ACT chain)
                    nc.vector.scalar_tensor_tensor(
                        a_ring[g][:, asl * gw:(asl + 1) * gw],
                        ring[0:N, slot * bl + c0:slot * bl + c1],
                        float(DECAY),
                        nzt[:, tt * bl + c0:tt * bl + c1],
                        mult, add)
                    # a_t = 0.85*DT*th_{t-1} + q_t -> next ring slot
                    pths = (t - 1) % th_slots
                    nc.vector.scalar_tensor_tensor(
                        ring[0:N, nslot * bl + c0:nslot * bl + c1],
                        th_sb[g][0:N, pths * gw:(pths + 1) * gw],
                        float(DECAY * DT),
                        a_ring[g][:, asl * gw:(asl + 1) * gw],
                        mult, add)

        # Output drain: retire a z region once its last slot is written.
        if t % z_slots == z_slots - 1 or t == n_steps - 1:
            r_idx = t // z_slots
            nslots = (t % z_slots) + 1
            for g in range(groups):
                gw = gws[g]
                zp = zps[g][reg]
                # compute-engine APs must start on a 32-aligned
                # partition: copy rows 96:103, DMA out only 100:103
                stg = spool.tile([K - 96, z_slots * gw], F32, tag=f"st{g}",
                                 name=f"stg{g}_{r_idx}")
                src = zp[96:K, 0:nslots * gw]
                dst = stg[:, 0:nslots * gw]
                # always DVE: the ACT engine is on the 2-hop critical chain
                nc.vector.tensor_copy(dst, src)
                # DRAM layout: col block j (width bl) = W_out @ h_j;
                # group g owns cols j*bl + [gos[g], gos[g]+gw).
                base = (r_idx * z_slots)
                nc.sync.dma_start(
                    aps["out_t"].rearrange("p (t b) -> p t b", b=bl)[
                        :, base:base + nslots, gos[g]:gos[g] + gw],
                    stg.rearrange("p (t b) -> p t b", b=gw)[
                        N - 96:K - 96, 0:nslots, :])

    for p in (ppool, spool, npool, rpool, cpool):
        p.release()


def build_nc(*, t_steps=T, bl=BL, groups=2, tc_chunk=128, ring_slots=128,
             z_slots=16, th_slots=8, a_slots=4, num_devices=NCORES,
             th_psum=True):
    nc = bacc.Bacc("TRN2", target_bir_lowering=False, debug=False,
                   num_devices=num_devices)
    aps = {
        "s_mat": nc.dram_tensor("s_mat", [KP, KP], F16,
                                kind="ExternalInput").ap(),
        "s2_mat": nc.dram_tensor("s2_mat", [KP, KP], F16,
                                 kind="ExternalInput").ap(),
        "noise_t": nc.dram_tensor("noise_t", [N, t_steps * bl], F32,
                                  kind="ExternalInput").ap(),
        "u_t": nc.dram_tensor("u_t", [NB, t_steps * bl], F16,
                              kind="ExternalInput").ap(),
        "out_t": nc.dram_tensor("out_t", [NB, (t_steps + 1) * bl], F32,
                                kind="ExternalOutput").ap(),
    }
    with tile.TileContext(nc) as tcx:
        emit_rnn(tcx, nc, aps, t_steps=t_steps, bl=bl, groups=groups,
                 tc_chunk=tc_chunk, ring_slots=ring_slots, z_slots=z_slots,
                 th_slots=th_slots, a_slots=a_slots, th_psum=th_psum)
    nc.compile()
    return nc


def make_s_mat(recurrent_weights, input_weights, output_weights):
    st = np.zeros((K, K), np.float32)
    st[:N, :N] = recurrent_weights
    st[:N, N:] = input_weights
    st[N:, :N] = output_weights
    sp = np.zeros((KP, KP), np.float16)
    sp[:K, :K] = st.T.astype(np.float16)
    return np.ascontiguousarray(sp)


def make_s2_mat(recurrent_weights, output_weights):
    st = np.zeros((K, K), np.float32)
    st[:N, :N] = recurrent_weights
    st[N:, :N] = output_weights
    sp = np.zeros((KP, KP), np.float16)
    sp[:K, :K] = (DT * st).T.astype(np.float16)
    return np.ascontiguousarray(sp)


def make_in_maps(inputs, noise, recurrent_weights, input_weights,
                 output_weights, *, t_steps=T, bl=BL, ncores=NCORES):
    s = make_s_mat(recurrent_weights, input_weights, output_weights)
    s2 = make_s2_mat(recurrent_weights, output_weights)
    in_maps = []
    for c in range(ncores):
        bs = slice(c * bl, (c + 1) * bl)
        nt = (noise[bs].astype(np.float32).transpose(2, 1, 0)
              * NOISE_STD).reshape(N, t_steps * bl)
        ut = np.ascontiguousarray(
            inputs[bs].astype(np.float16).transpose(2, 1, 0)
        ).reshape(NB, t_steps * bl)
        in_maps.append({"s_mat": s, "s2_mat": s2,
                        "noise_t": np.ascontiguousarray(nt),
                        "u_t": ut})
    return in_maps


def gather_out(results, *, t_steps=T, bl=BL, ncores=NCORES):
    out = np.empty((ncores * bl, t_steps, NB), np.float32)
    for c in range(ncores):
        ot = results[c]["out_t"].reshape(NB, t_steps + 1, bl)
        out[c * bl:(c + 1) * bl] = ot[:, 1:, :].transpose(2, 1, 0)
    return out


_NC_CACHE = {}


def kernel(inputs, noise, recurrent_weights, input_weights, output_weights,
           **run_kwargs):
    cfg = run_kwargs.pop("cfg", {})
    key = tuple(sorted(cfg.items()))
    if key not in _NC_CACHE:
        _NC_CACHE[key] = build_nc(**cfg)
    nc = _NC_CACHE[key]
    in_maps = make_in_maps(inputs, noise, recurrent_weights, input_weights,
                           output_weights)
    res = run_bass_kernel_spmd(nc, in_maps, core_ids=list(range(NCORES)),
                               **run_kwargs)
    out = gather_out(res.results)
    if run_kwargs.get("trace"):
        return out, res
    return out
